# revision 2
# baseline (speedup 1.0000x reference)
"""DWT-attention Trainium2 kernel.

Math (per batch b, all on device):
  out = ( iDWT3( W ⊙ (DWT3(x)Wq^T) ⊙ (DWT3(x)Wv^T) ) ) Wout^T + b_out
using that the Haar DWT along L commutes with channel projections, so the
DWT is applied ONCE to x (not to q and v separately), and all 1/sqrt(2)
band scales + the per-(head,channel) band weights are folded into
per-channel scalars applied on the PSUM->SBUF copy.

Layout: everything transposed — x^T [D, L] so L is the free dim (DWT is
free-dim strided adds on DVE) and channels are partitions (band weights
become per-partition scalars). Sharding: batch B=8, one batch per core.
Matmuls run in float32r (fast-FP32 PE mode, ~1e-4 rel err).
"""
import sys
sys.path.insert(0, "/opt/trn_rl_repo")
import numpy as np

B, L, D, H, NMODE, Dh = 8, 4096, 1024, 16, 3, 64
LC = 1024                  # L-chunk
NCHUNK = L // LC
KT = D // 128              # k tiles (contraction)
MT = D // 128              # m tiles (output channels)
INVSQRT2 = 0.7071067811865476

_CACHE = {}


def _build():
    import concourse.bacc as bacc
    import concourse.mybir as mybir
    import concourse.tile as tile

    f32, f32r = mybir.dt.float32, mybir.dt.float32r
    ident = mybir.ActivationFunctionType.Identity
    add_op, mult_op = mybir.AluOpType.add, mybir.AluOpType.mult
    C3 = INVSQRT2 ** 3

    nc = bacc.Bacc("TRN2", target_bir_lowering=False, debug=False)
    xt = nc.dram_tensor("xt", [D, L], f32, kind="ExternalInput")
    wqt = nc.dram_tensor("wqt", [D, D], f32r, kind="ExternalInput")
    wvt = nc.dram_tensor("wvt", [D, D], f32r, kind="ExternalInput")
    wot = nc.dram_tensor("wot", [D, D], f32r, kind="ExternalInput")
    scal = nc.dram_tensor("scal", [128, 48], f32, kind="ExternalInput")
    outt = nc.dram_tensor("outt", [D, L], f32, kind="ExternalOutput")

    with tile.TileContext(nc) as tc:
        with tc.tile_pool(name="wq", bufs=KT) as wq_pool, \
             tc.tile_pool(name="wv", bufs=KT) as wv_pool, \
             tc.tile_pool(name="wo", bufs=KT) as wo_pool, \
             tc.tile_pool(name="const", bufs=1) as const_pool, \
             tc.tile_pool(name="x", bufs=2) as x_pool, \
             tc.tile_pool(name="u", bufs=8) as u_pool, \
             tc.tile_pool(name="ta", bufs=2) as ta_pool, \
             tc.tile_pool(name="cq", bufs=3) as cq_pool, \
             tc.tile_pool(name="prod", bufs=2) as prod_pool, \
             tc.tile_pool(name="ti", bufs=2) as ti_pool, \
             tc.tile_pool(name="y", bufs=8) as y_pool, \
             tc.tile_pool(name="o", bufs=4) as o_pool, \
             tc.tile_pool(name="psum", bufs=6, space="PSUM") as psum_pool:

            scal_sb = const_pool.tile([128, 48], f32)
            nc.sync.dma_start(scal_sb[:], scal.ap())

            def sap(j, m):          # per-partition scalar column
                return scal_sb[:, j * 8 + m: j * 8 + m + 1]

            wq_t, wv_t, wo_t = [], [], []
            for k in range(KT):
                sl = slice(k * 128, (k + 1) * 128)
                t = wq_pool.tile([128, D], f32r, tag="wq")
                nc.sync.dma_start(t[:], wqt.ap()[sl, :])
                wq_t.append(t)
                t = wv_pool.tile([128, D], f32r, tag="wv")
                nc.sync.dma_start(t[:], wvt.ap()[sl, :])
                wv_t.append(t)
                t = wo_pool.tile([128, D], f32r, tag="wo")
                nc.sync.dma_start(t[:], wot.ap()[sl, :])
                wo_t.append(t)

            for c in range(NCHUNK):
                csl = slice(c * LC, (c + 1) * LC)
                # ---- stage A: unscaled Haar-3 DWT of x^T chunk ----
                u_c = []
                for k in range(KT):
                    xt_t = x_pool.tile([128, LC], f32, tag="x")
                    nc.sync.dma_start(xt_t[:], xt.ap()[k * 128:(k + 1) * 128, csl])
                    ut = u_pool.tile([128, LC], f32r, tag="u")
                    e, o = xt_t[:, 0:LC:2], xt_t[:, 1:LC:2]
                    nc.vector.tensor_sub(ut[:, 512:1024], e, o)          # D1
                    t1 = ta_pool.tile([128, 512], f32, tag="t1")
                    nc.vector.tensor_add(t1[:], e, o)                    # A1
                    e2, o2 = t1[:, 0:512:2], t1[:, 1:512:2]
                    nc.vector.tensor_sub(ut[:, 256:512], e2, o2)         # D2
                    t2 = ta_pool.tile([128, 256], f32, tag="t2")
                    nc.vector.tensor_add(t2[:], e2, o2)                  # A2
                    e3, o3 = t2[:, 0:256:2], t2[:, 1:256:2]
                    nc.vector.tensor_sub(ut[:, 128:256], e3, o3)         # D3
                    nc.vector.tensor_add(ut[:, 0:128], e3, o3)           # A3
                    u_c.append(ut)

                # ---- stages B (project+weight+product) and C (iDWT) ----
                y_c = []
                for m in range(MT):
                    msl = slice(m * 128, (m + 1) * 128)
                    prod = prod_pool.tile([128, LC], f32, tag="prod")
                    for hf in range(2):
                        nsl = slice(hf * 512, (hf + 1) * 512)
                        psq = psum_pool.tile([128, 512], f32, tag="ps")
                        for k in range(KT):
                            nc.tensor.matmul(psq[:], wq_t[k][:, msl], u_c[k][:, nsl],
                                             start=(k == 0), stop=(k == KT - 1))
                        psv = psum_pool.tile([128, 512], f32, tag="ps")
                        for k in range(KT):
                            nc.tensor.matmul(psv[:], wv_t[k][:, msl], u_c[k][:, nsl],
                                             start=(k == 0), stop=(k == KT - 1))
                        cq = cq_pool.tile([128, 512], f32, tag="cq")
                        if hf == 0:
                            # bands [A3 0:128 | D3 128:256 | D2 256:512]
                            nc.scalar.activation(cq[:, 0:128], psq[:, 0:128], ident,
                                                 bias=sap(3, m), scale=sap(0, m))
                            nc.scalar.mul(cq[:, 128:256], psq[:, 128:256], sap(1, m))
                            nc.scalar.mul(cq[:, 256:512], psq[:, 256:512], sap(2, m))
                            nc.vector.scalar_tensor_tensor(
                                prod[:, 0:128], psv[:, 0:128], sap(4, m), cq[:, 0:128],
                                op0=add_op, op1=mult_op)
                            nc.vector.tensor_mul(prod[:, 128:512], cq[:, 128:512],
                                                 psv[:, 128:512])
                        else:
                            # band D1, constant scale c^3
                            nc.scalar.mul(cq[:], psq[:], C3)
                            nc.vector.tensor_mul(prod[:, 512:1024], cq[:], psv[:])
                    # iDWT (unscaled butterflies, interleaved writes)
                    it2 = ti_pool.tile([128, 256], f32, tag="it2")
                    nc.vector.tensor_add(it2[:, 0:256:2], prod[:, 0:128], prod[:, 128:256])
                    nc.vector.tensor_sub(it2[:, 1:256:2], prod[:, 0:128], prod[:, 128:256])
                    it1 = ti_pool.tile([128, 512], f32, tag="it1")
                    nc.vector.tensor_add(it1[:, 0:512:2], it2[:], prod[:, 256:512])
                    nc.vector.tensor_sub(it1[:, 1:512:2], it2[:], prod[:, 256:512])
                    yt = y_pool.tile([128, LC], f32r, tag="y")
                    nc.vector.tensor_add(yt[:, 0:LC:2], it1[:], prod[:, 512:1024])
                    nc.vector.tensor_sub(yt[:, 1:LC:2], it1[:], prod[:, 512:1024])
                    y_c.append(yt)

                # ---- stage D: output projection ----
                for m in range(MT):
                    msl = slice(m * 128, (m + 1) * 128)
                    for hf in range(2):
                        nsl = slice(hf * 512, (hf + 1) * 512)
                        pso = psum_pool.tile([128, 512], f32, tag="ps")
                        for k in range(KT):
                            nc.tensor.matmul(pso[:], wo_t[k][:, msl], y_c[k][:, nsl],
                                             start=(k == 0), stop=(k == KT - 1))
                        ot = o_pool.tile([128, 512], f32, tag="o")
                        nc.scalar.activation(ot[:], pso[:], ident, bias=sap(5, m), scale=1.0)
                        nc.sync.dma_start(
                            outt.ap()[msl, c * LC + hf * 512: c * LC + (hf + 1) * 512],
                            ot[:])
    nc.compile()
    return nc


def _prep(W_qkv, b_qkv, W_out, b_out, weight_q, weight_v):
    c = INVSQRT2
    WqT = np.ascontiguousarray(W_qkv[0:D, :].T, dtype=np.float32)
    WvT = np.ascontiguousarray(W_qkv[2 * D:3 * D, :].T, dtype=np.float32)
    WoT = np.ascontiguousarray(W_out.T, dtype=np.float32)
    wb = np.empty((3, D), np.float32)
    for j in range(3):
        wb[j] = (np.asarray(weight_q)[:, j, :] * np.asarray(weight_v)[:, j, :]).reshape(D)
    wb0 = wb[0] * c ** 9
    wb1 = wb[1] * c ** 9
    wb2 = wb[2] * c ** 6
    bq = np.asarray(b_qkv)[0:D].astype(np.float32)
    bv = np.asarray(b_qkv)[2 * D:3 * D].astype(np.float32)
    bqw8 = wb0 * 8.0 * bq
    bv8 = 8.0 * bv
    bout = np.asarray(b_out).astype(np.float32)
    scal = np.empty((128, 48), np.float32)
    for j, vec in enumerate((wb0, wb1, wb2, bqw8, bv8, bout)):
        scal[:, j * 8:(j + 1) * 8] = vec.reshape(8, 128).T
    return WqT, WvT, WoT, scal


def kernel(query, W_qkv, b_qkv, W_out, b_out, weight_q, weight_v, _trace=False):
    from concourse.bass_utils import run_bass_kernel_spmd

    if "nc" not in _CACHE:
        _CACHE["nc"] = _build()
    nc = _CACHE["nc"]

    WqT, WvT, WoT, scal = _prep(W_qkv, b_qkv, W_out, b_out, weight_q, weight_v)
    query = np.asarray(query, dtype=np.float32)
    in_maps = []
    for b in range(B):
        in_maps.append({
            "xt": np.ascontiguousarray(query[b].T),
            "wqt": WqT, "wvt": WvT, "wot": WoT, "scal": scal,
        })
    res = run_bass_kernel_spmd(nc, in_maps, list(range(B)), trace=_trace)
    out = np.empty((B, L, D), np.float32)
    for b in range(B):
        out[b] = res.results[b]["outt"].T
    if _trace:
        _CACHE["last_results"] = res
    return out


# revision 7
# speedup vs baseline: 1.0514x; 1.0514x over previous
"""DWT-attention Trainium2 kernel.

Math (per batch b, all on device):
  out = ( iDWT3( W ⊙ (DWT3(x)Wq^T) ⊙ (DWT3(x)Wv^T) ) ) Wout^T + b_out
using that the Haar DWT along L commutes with channel projections, so the
DWT is applied ONCE to x (not to q and v separately), and all 1/sqrt(2)
band scales + the per-(head,channel) band weights are folded into
per-channel scalars applied on the PSUM->SBUF copy.

Layout: everything transposed — x^T [D, L] so L is the free dim (DWT is
free-dim strided adds on DVE) and channels are partitions (band weights
become per-partition scalars). Sharding: batch B=8, one batch per core.
Matmuls run in float32r (fast-FP32 PE mode, ~1e-4 rel err).
"""
import sys
sys.path.insert(0, "/opt/trn_rl_repo")
import numpy as np

B, L, D, H, NMODE, Dh = 8, 4096, 1024, 16, 3, 64
LC = 1024                  # L-chunk
NCHUNK = L // LC
KT = D // 128              # k tiles (contraction)
MT = D // 128              # m tiles (output channels)
INVSQRT2 = 0.7071067811865476

_CACHE = {}


def _build():
    import concourse.bacc as bacc
    import concourse.mybir as mybir
    import concourse.tile as tile

    f32, f32r = mybir.dt.float32, mybir.dt.float32r
    ident = mybir.ActivationFunctionType.Identity
    add_op, mult_op = mybir.AluOpType.add, mybir.AluOpType.mult
    C3 = INVSQRT2 ** 3

    nc = bacc.Bacc("TRN2", target_bir_lowering=False, debug=False)
    xt = nc.dram_tensor("xt", [D, L], f32, kind="ExternalInput")
    wqt = nc.dram_tensor("wqt", [D, D], f32r, kind="ExternalInput")
    wvt = nc.dram_tensor("wvt", [D, D], f32r, kind="ExternalInput")
    wot = nc.dram_tensor("wot", [D, D], f32r, kind="ExternalInput")
    scal = nc.dram_tensor("scal", [128, 48], f32, kind="ExternalInput")
    outt = nc.dram_tensor("outt", [D, L], f32, kind="ExternalOutput")

    with tile.TileContext(nc) as tc:
        with tc.tile_pool(name="wq", bufs=KT) as wq_pool, \
             tc.tile_pool(name="wv", bufs=KT) as wv_pool, \
             tc.tile_pool(name="wo", bufs=KT) as wo_pool, \
             tc.tile_pool(name="const", bufs=1) as const_pool, \
             tc.tile_pool(name="x", bufs=2) as x_pool, \
             tc.tile_pool(name="u", bufs=8) as u_pool, \
             tc.tile_pool(name="ta", bufs=2) as ta_pool, \
             tc.tile_pool(name="cq", bufs=2) as cq_pool, \
             tc.tile_pool(name="prod", bufs=2) as prod_pool, \
             tc.tile_pool(name="ti", bufs=2) as ti_pool, \
             tc.tile_pool(name="y", bufs=8) as y_pool, \
             tc.tile_pool(name="o", bufs=2) as o_pool, \
             tc.tile_pool(name="psum", bufs=8, space="PSUM") as psum_pool:

            scal_sb = const_pool.tile([128, 48], f32)
            nc.sync.dma_start(scal_sb[:], scal.ap())

            def sap(j, m):          # per-partition scalar column
                return scal_sb[:, j * 8 + m: j * 8 + m + 1]

            def dwt_tile(k, csl):
                """Load x^T tile and produce its unscaled Haar-3 bands."""
                xt_t = x_pool.tile([128, LC], f32, tag="x")
                nc.sync.dma_start(xt_t[:], xt.ap()[k * 128:(k + 1) * 128, csl])
                ut = u_pool.tile([128, LC], f32r, tag="u")
                e, o = xt_t[:, 0:LC:2], xt_t[:, 1:LC:2]
                nc.vector.tensor_sub(ut[:, 512:1024], e, o)          # D1
                t1 = ta_pool.tile([128, 512], f32, tag="t1")
                nc.vector.tensor_add(t1[:], e, o)                    # A1
                e2, o2 = t1[:, 0:512:2], t1[:, 1:512:2]
                nc.vector.tensor_sub(ut[:, 256:512], e2, o2)         # D2
                t2 = ta_pool.tile([128, 256], f32, tag="t2")
                nc.vector.tensor_add(t2[:], e2, o2)                  # A2
                e3, o3 = t2[:, 0:256:2], t2[:, 1:256:2]
                nc.vector.tensor_sub(ut[:, 128:256], e3, o3)         # D3
                nc.vector.tensor_add(ut[:, 0:128], e3, o3)           # A3
                return ut

            # startup order: x chunk-0 + its DWT first so PE can start as
            # soon as wq arrives; wo (needed only by stage D) loads last.
            u_first = [dwt_tile(k, slice(0, LC)) for k in range(KT)]

            wq_t, wv_t, wo_t = [], [], []
            for k in range(KT):
                sl = slice(k * 128, (k + 1) * 128)
                t = wq_pool.tile([128, D], f32r, tag="wq")
                nc.sync.dma_start(t[:], wqt.ap()[sl, :])
                wq_t.append(t)
            for k in range(KT):
                sl = slice(k * 128, (k + 1) * 128)
                t = wv_pool.tile([128, D], f32r, tag="wv")
                nc.sync.dma_start(t[:], wvt.ap()[sl, :])
                wv_t.append(t)
            for k in range(KT):
                sl = slice(k * 128, (k + 1) * 128)
                t = wo_pool.tile([128, D], f32r, tag="wo")
                nc.sync.dma_start(t[:], wot.ap()[sl, :])
                wo_t.append(t)

            for c in range(NCHUNK):
                csl = slice(c * LC, (c + 1) * LC)
                # ---- stage A: unscaled Haar-3 DWT of x^T chunk ----
                if c == 0:
                    u_c = u_first
                else:
                    u_c = [dwt_tile(k, csl) for k in range(KT)]

                # ---- stages B (project+weight+product) and C (iDWT) ----
                y_c = []
                for m in range(MT):
                    msl = slice(m * 128, (m + 1) * 128)
                    prod = prod_pool.tile([128, LC], f32, tag="prod")
                    # paired n-halves per stationary block (one LDW, 2 MMs)
                    psq0 = psum_pool.tile([128, 512], f32, tag="ps")
                    psq1 = psum_pool.tile([128, 512], f32, tag="ps")
                    for k in range(KT):
                        nc.tensor.matmul(psq0[:], wq_t[k][:, msl], u_c[k][:, 0:512],
                                         start=(k == 0), stop=(k == KT - 1))
                        nc.tensor.matmul(psq1[:], wq_t[k][:, msl], u_c[k][:, 512:1024],
                                         start=(k == 0), stop=(k == KT - 1))
                    psv0 = psum_pool.tile([128, 512], f32, tag="ps")
                    psv1 = psum_pool.tile([128, 512], f32, tag="ps")
                    for k in range(KT):
                        nc.tensor.matmul(psv0[:], wv_t[k][:, msl], u_c[k][:, 0:512],
                                         start=(k == 0), stop=(k == KT - 1))
                        nc.tensor.matmul(psv1[:], wv_t[k][:, msl], u_c[k][:, 512:1024],
                                         start=(k == 0), stop=(k == KT - 1))
                    cq = cq_pool.tile([128, 1024], f32, tag="cq")
                    # bands [A3 0:128 | D3 128:256 | D2 256:512 | D1 512:1024]
                    nc.scalar.activation(cq[:, 0:128], psq0[:, 0:128], ident,
                                         bias=sap(3, m), scale=sap(0, m))
                    nc.scalar.mul(cq[:, 128:256], psq0[:, 128:256], sap(1, m))
                    nc.scalar.mul(cq[:, 256:512], psq0[:, 256:512], sap(2, m))
                    nc.scalar.mul(cq[:, 512:1024], psq1[:], C3)
                    nc.vector.scalar_tensor_tensor(
                        prod[:, 0:128], psv0[:, 0:128], sap(4, m), cq[:, 0:128],
                        op0=add_op, op1=mult_op)
                    nc.vector.tensor_mul(prod[:, 128:512], cq[:, 128:512],
                                         psv0[:, 128:512])
                    nc.vector.tensor_mul(prod[:, 512:1024], cq[:, 512:1024], psv1[:])
                    # iDWT (unscaled butterflies, interleaved writes)
                    it2 = ti_pool.tile([128, 256], f32, tag="it2")
                    nc.vector.tensor_add(it2[:, 0:256:2], prod[:, 0:128], prod[:, 128:256])
                    nc.vector.tensor_sub(it2[:, 1:256:2], prod[:, 0:128], prod[:, 128:256])
                    it1 = ti_pool.tile([128, 512], f32, tag="it1")
                    nc.vector.tensor_add(it1[:, 0:512:2], it2[:], prod[:, 256:512])
                    nc.vector.tensor_sub(it1[:, 1:512:2], it2[:], prod[:, 256:512])
                    yt = y_pool.tile([128, LC], f32r, tag="y")
                    nc.vector.tensor_add(yt[:, 0:LC:2], it1[:], prod[:, 512:1024])
                    nc.vector.tensor_sub(yt[:, 1:LC:2], it1[:], prod[:, 512:1024])
                    y_c.append(yt)

                # ---- stage D: output projection (paired n-halves) ----
                for m in range(MT):
                    msl = slice(m * 128, (m + 1) * 128)
                    pso0 = psum_pool.tile([128, 512], f32, tag="ps")
                    pso1 = psum_pool.tile([128, 512], f32, tag="ps")
                    for k in range(KT):
                        nc.tensor.matmul(pso0[:], wo_t[k][:, msl], y_c[k][:, 0:512],
                                         start=(k == 0), stop=(k == KT - 1))
                        nc.tensor.matmul(pso1[:], wo_t[k][:, msl], y_c[k][:, 512:1024],
                                         start=(k == 0), stop=(k == KT - 1))
                    ot = o_pool.tile([128, 1024], f32, tag="o")
                    nc.scalar.activation(ot[:, 0:512], pso0[:], ident,
                                         bias=sap(5, m), scale=1.0)
                    nc.scalar.activation(ot[:, 512:1024], pso1[:], ident,
                                         bias=sap(5, m), scale=1.0)
                    nc.sync.dma_start(outt.ap()[msl, csl], ot[:])
    nc.compile()
    return nc


def _prep(W_qkv, b_qkv, W_out, b_out, weight_q, weight_v):
    c = INVSQRT2
    WqT = np.ascontiguousarray(W_qkv[0:D, :].T, dtype=np.float32)
    WvT = np.ascontiguousarray(W_qkv[2 * D:3 * D, :].T, dtype=np.float32)
    WoT = np.ascontiguousarray(W_out.T, dtype=np.float32)
    wb = np.empty((3, D), np.float32)
    for j in range(3):
        wb[j] = (np.asarray(weight_q)[:, j, :] * np.asarray(weight_v)[:, j, :]).reshape(D)
    wb0 = wb[0] * c ** 9
    wb1 = wb[1] * c ** 9
    wb2 = wb[2] * c ** 6
    bq = np.asarray(b_qkv)[0:D].astype(np.float32)
    bv = np.asarray(b_qkv)[2 * D:3 * D].astype(np.float32)
    bqw8 = wb0 * 8.0 * bq
    bv8 = 8.0 * bv
    bout = np.asarray(b_out).astype(np.float32)
    scal = np.empty((128, 48), np.float32)
    for j, vec in enumerate((wb0, wb1, wb2, bqw8, bv8, bout)):
        scal[:, j * 8:(j + 1) * 8] = vec.reshape(8, 128).T
    return WqT, WvT, WoT, scal


def kernel(query, W_qkv, b_qkv, W_out, b_out, weight_q, weight_v, _trace=False):
    from concourse.bass_utils import run_bass_kernel_spmd

    if "nc" not in _CACHE:
        _CACHE["nc"] = _build()
    nc = _CACHE["nc"]

    WqT, WvT, WoT, scal = _prep(W_qkv, b_qkv, W_out, b_out, weight_q, weight_v)
    query = np.asarray(query, dtype=np.float32)
    in_maps = []
    for b in range(B):
        in_maps.append({
            "xt": np.ascontiguousarray(query[b].T),
            "wqt": WqT, "wvt": WvT, "wot": WoT, "scal": scal,
        })
    res = run_bass_kernel_spmd(nc, in_maps, list(range(B)), trace=_trace)
    out = np.empty((B, L, D), np.float32)
    for b in range(B):
        out[b] = res.results[b]["outt"].T
    if _trace:
        _CACHE["last_results"] = res
    return out
